# revision 33
# baseline (speedup 1.0000x reference)
"""Trainium2 Bass kernel for nn_Net_24077586661451 (12-layer Mamba, d_model=70).

Sharding: 8 cores = 2 samples x 4 e-chunks (ED=140 -> 35/core); params
replicated, one AllGather of y per layer over each 4-core group.
Per-core scan grid: 560 partitions (35 e x 16 n, e-major p = e*16+n) as 5
partition tiles (4x128 + 48). L = 2048 = 4 chunks of Q=512 (PSUM free size).

vs the 5.5ms v0: all matmuls bf16 (fp32 PE is 4 cyc/row, bf16 is 1);
full-L (FD=2048) tensor_tensor_scan per grid tile, k-pipelined with the
dA/dBx production; B/C produced pre-replicated straight from xi (B_rep row
p = x_proj row DTR + p%16, identical for every grid tile); softplus =
exp + 3-term ln1p series on DVE (input <= -1.8 so t <= 0.17); rmsnorm
rsqrt = exp(-0.5 ln(mean)); activation blocks gated through tiny DVE-made
scale operands so the act table loads ~4x/layer instead of thrashing on
the scheduler's readiness-order interleaving; one AllGather per layer
(split AGs and DMA-broadcast variants both measured slower).

Layer pipeline (per layer):
  S1 rmsnorm: h^2 (DVE), ones-matmul (PE), Ln / Exp blocks (ACT) -> hsc bf16
  S2 conv-fused in_proj (4 shifted-tap PE matmuls) + z proj, Silu (ACT)
  S3 dt (PE, premult dt_w@x_proj), exp (ACT) + ln1p series (DVE) -> delta,
     u = delta*xi (GP); B_rep/C_rep (PE) + copies (ACT) -> full-L SBUF
  S4-S6 per grid tile k: delta/u selector-matmuls -> PSUM (PE),
     dA = exp(A*delta_b) (ACT), dBx = u_b*B_rep (DVE),
     tensor_tensor_scan FD=2048 (DVE), hc = h*C_rep (DVE),
     n-reduce matmuls -> y PSUM (PE)
  S7 gate: D*xi+y (DVE stt), *silu(z) (DVE), DMA chunks to DRAM
  S8 AllGather y over the 4-core group (DRAM bounce)
  S9 out_proj (PE) + residual add (DVE f32)

Each core's xi channel order is permuted so its own 35 channels are rows 0:35
(weights permuted host-side; the program is identical across cores - SPMD).
"""
import ml_dtypes
import numpy as np

import concourse.bass as bass
import concourse.bacc as bacc
import concourse.mybir as mybir
import concourse.tile as tile
from concourse.bass_utils import run_bass_kernel_spmd

f32 = mybir.dt.float32
bf16 = mybir.dt.bfloat16
AF = mybir.ActivationFunctionType
OP = mybir.AluOpType

B, L, IN_DIM, D, ED, N, NL, DTR = 2, 2048, 32, 70, 140, 16, 12, 5
E = ED // 4                      # 35 channels per core
NCORES, GROUP = 8, 4
Q = 512
NCH = L // Q
EPS = 1e-5
# grid partition tiles: (pstart, pcount); p = e_loc*16 + n
GTILES = [(0, 128), (128, 128), (256, 128), (384, 128), (512, 48)]

_CACHE = {}


def _build_nc():
    nc = bacc.Bacc("TRN2", target_bir_lowering=False, debug=False)

    di = {}

    def dram_in(name, shape, dt=f32):
        di[name] = nc.dram_tensor(name, list(shape), dt, kind="ExternalInput")
        return di[name]

    dram_in("x_t", (IN_DIM, L))
    dram_in("w_in", (IN_DIM, D))
    dram_in("b_in", (D, 1))
    dram_in("taps", (D, NL * 4 * ED), bf16)
    dram_in("zw", (D, NL * E), bf16)
    dram_in("brepA", (128, NL * 128), bf16)
    dram_in("brepB", (12, NL * 128), bf16)
    dram_in("crepA", (128, NL * 128), bf16)
    dram_in("crepB", (12, NL * 128), bf16)
    dram_in("dtwA", (128, NL * E), bf16)
    dram_in("dtwB", (12, NL * E), bf16)
    dram_in("outwA", (128, NL * D), bf16)
    dram_in("outwB", (12, NL * D), bf16)
    dram_in("dtb", (E, NL))
    dram_in("cbA", (128, NL))
    dram_in("cbB", (12, NL))
    dram_in("dpv", (E, NL))
    dram_in("asc", (128, NL * 5))
    dram_in("seld", (E, 5 * 128), bf16)
    dram_in("red", (128, 5 * E), bf16)
    dram_in("ones70", (D, 1), bf16)
    dram_in("ones1", (1, D), bf16)
    dram_in("wout", (D, 1))
    dram_in("bout", (1, 1))
    out_d = nc.dram_tensor("out", [1, L], f32, kind="ExternalOutput")

    with tile.TileContext(nc) as tc:
        with (
            tc.tile_pool(name="wts", bufs=1) as wts,
            tc.tile_pool(name="hbuf", bufs=1) as hbuf,
            tc.tile_pool(name="fl", bufs=1) as fl,           # full-L per layer
            tc.tile_pool(name="gr", bufs=1) as gr,           # grid full-L
            tc.tile_pool(name="sm", bufs=3) as sm,           # per-chunk small
            tc.tile_pool(name="ps_a", bufs=3, space="PSUM") as ps_a,
            tc.tile_pool(name="ps_y", bufs=4, space="PSUM") as ps_y,
            tc.tile_pool(name="ps_s", bufs=1, space="PSUM") as ps_s,
            tc.tile_pool(name="dr", bufs=2, space="DRAM") as dr,
        ):
            wt = {}
            for name, h in di.items():
                t = wts.tile(list(h.shape), h.dtype, tag=f"w_{name}")
                nc.sync.dma_start(t[:], h[:])
                wt[name] = t

            # persistent activation buffers
            h_a = hbuf.tile([D, L], f32)
            h_b = hbuf.tile([D, L], f32)
            hsc = hbuf.tile([D, L + 3], bf16)  # rms-scaled h, 3-col zero pad
            nc.vector.memset(hsc[:, 0:3], 0.0)

            # ---- embed: h_a = W_in @ x + b_in ----
            for c in range(NCH):
                sl = slice(c * Q, (c + 1) * Q)
                h0 = ps_a.tile([D, Q], f32, tag="psa")
                nc.tensor.matmul(h0[:], wt["w_in"][:], wt["x_t"][:, sl])
                nc.scalar.activation(h_a[:, sl], h0[:], AF.Identity,
                                     bias=wt["b_in"][:, 0:1], scale=1.0)

            h_cur, h_nxt = h_a, h_b

            for l in range(NL):
                # ================= S1: rmsnorm =================
                # rsqrt via exp(-0.5*ln(mean)); all Ln writes land in one
                # full-L tile so the Exp block only becomes ready after the
                # last Ln - whole-tile deps keep the act-table blocks grouped.
                rsf = fl.tile([1, L], bf16, tag="rsf")
                lnf = fl.tile([1, L], f32, tag="lnf")
                for c in range(NCH):
                    sl = slice(c * Q, (c + 1) * Q)
                    sq = sm.tile([D, Q], bf16, tag="sq", bufs=2)
                    nc.vector.tensor_tensor(sq[:], h_cur[:, sl], h_cur[:, sl],
                                            OP.mult)
                    ms = ps_s.tile([1, Q], f32, tag="pss")
                    nc.tensor.matmul(ms[:], wt["ones70"][:], sq[:])
                    nc.scalar.activation(lnf[:, sl], ms[:], AF.Ln)
                # scale operand built from the LAST Ln output: the whole Exp
                # block waits for the Ln block, so the act table loads once
                # per block instead of thrashing on ln<->exp alternation.
                g1 = sm.tile([1, 1], f32, tag="g1", bufs=2)
                nc.vector.tensor_scalar(g1[:], lnf[:, L - 1:L], 0.0, -0.5,
                                        OP.mult, OP.add)
                for c in range(NCH):
                    sl = slice(c * Q, (c + 1) * Q)
                    nc.scalar.activation(rsf[:, sl], lnf[:, sl], AF.Exp,
                                         scale=g1[:])
                for c in range(NCH):
                    sl = slice(c * Q, (c + 1) * Q)
                    rs70 = ps_a.tile([D, Q], f32, tag="psa")
                    nc.tensor.matmul(rs70[:], wt["ones1"][:], rsf[:, sl])
                    nc.vector.tensor_tensor(hsc[:, 3 + c * Q:3 + (c + 1) * Q],
                                            h_cur[:, sl], rs70[:], OP.mult)

                # ================= S2: in_proj taps + z =================
                xiA = fl.tile([128, L], bf16, tag="xiA")
                xiB = fl.tile([12, L], bf16, tag="xiB")
                zs = fl.tile([E, L], bf16, tag="zs")
                for c in range(NCH):
                    sl = slice(c * Q, (c + 1) * Q)
                    xa = ps_a.tile([128, Q], f32, tag="psa")
                    xb = ps_s.tile([12, Q], f32, tag="pss")
                    for k in range(4):
                        tap = wt["taps"][:, (l * 4 + k) * ED:(l * 4 + k + 1) * ED]
                        rhs = hsc[:, c * Q + k:c * Q + k + Q]
                        nc.tensor.matmul(xa[:], tap[:, 0:128], rhs,
                                         start=(k == 0), stop=(k == 3))
                        nc.tensor.matmul(xb[:], tap[:, 128:ED], rhs,
                                         start=(k == 0), stop=(k == 3))
                    zp = ps_s.tile([E, Q], f32, tag="pss")
                    nc.tensor.matmul(zp[:], wt["zw"][:, l * E:(l + 1) * E],
                                     hsc[:, 3 + c * Q:3 + (c + 1) * Q])
                    nc.scalar.activation(xiA[:, sl], xa[:], AF.Silu,
                                         bias=wt["cbA"][:, l:l + 1], scale=1.0)
                    nc.scalar.activation(xiB[:, sl], xb[:], AF.Silu,
                                         bias=wt["cbB"][:, l:l + 1], scale=1.0)
                    nc.scalar.activation(zs[:, sl], zp[:], AF.Silu)

                # ================= S3: delta (softplus), u, B/C reps =========
                delta = fl.tile([E, L], bf16, tag="delta")
                u = fl.tile([E, L], bf16, tag="u")
                dps, ezs = [], []
                for c in range(NCH):
                    sl = slice(c * Q, (c + 1) * Q)
                    dp = ps_a.tile([E, Q], f32, tag="psa")
                    nc.tensor.matmul(dp[:], wt["dtwA"][:, l * E:(l + 1) * E],
                                     xiA[:, sl], start=True, stop=False)
                    nc.tensor.matmul(dp[:], wt["dtwB"][:, l * E:(l + 1) * E],
                                     xiB[:, sl], start=False, stop=True)
                    dps.append(dp)
                g2 = sm.tile([E, 1], f32, tag="g2", bufs=2)
                nc.vector.tensor_scalar(g2[:], zs[:, L - 1:L], 0.0, 1.0,
                                        OP.mult, OP.add)
                for c in range(NCH):
                    ez = sm.tile([E, Q], bf16, tag="ez", bufs=3)
                    nc.scalar.activation(ez[:], dps[c][:], AF.Exp,
                                         bias=wt["dtb"][:, l:l + 1],
                                         scale=g2[:])
                    ezs.append(ez)
                for c in range(NCH):
                    # softplus(x) = ln(1+t), t=exp(x) <= 0.17 here (dt_b in
                    # [-4,-2]): 3-term series t*(1 - t*(0.5 - t/3)) on DVE
                    # keeps ln off the scalar engine (act-table thrash).
                    sl = slice(c * Q, (c + 1) * Q)
                    t = ezs[c]
                    p1 = sm.tile([E, Q], bf16, tag="p1", bufs=2)
                    nc.vector.tensor_scalar(p1[:], t[:], -1.0 / 3, 0.5,
                                            OP.mult, OP.add)
                    p2 = sm.tile([E, Q], bf16, tag="p2", bufs=2)
                    nc.vector.tensor_tensor(p2[:], t[:], p1[:], OP.mult)
                    p3 = sm.tile([E, Q], bf16, tag="p1", bufs=2, name="p3")
                    nc.vector.tensor_scalar(p3[:], p2[:], -1.0, 1.0,
                                            OP.mult, OP.add)
                    nc.vector.tensor_tensor(delta[:, sl], t[:], p3[:], OP.mult)
                    nc.gpsimd.tensor_tensor(u[:, sl], delta[:, sl],
                                            xiA[0:E, sl], OP.mult)
                dps = ezs = None
                # B_rep[p,t] = B[p%16,t] identical for every grid tile; full-L
                Bbs = fl.tile([128, L], bf16, tag="Bbs")
                Cbs = fl.tile([128, L], bf16, tag="Cbs")
                for c in range(NCH):
                    sl = slice(c * Q, (c + 1) * Q)
                    Bp = ps_a.tile([128, Q], f32, tag="psa")
                    nc.tensor.matmul(Bp[:], wt["brepA"][:, l * 128:(l + 1) * 128],
                                     xiA[:, sl], start=True, stop=False)
                    nc.tensor.matmul(Bp[:], wt["brepB"][:, l * 128:(l + 1) * 128],
                                     xiB[:, sl], start=False, stop=True)
                    nc.scalar.copy(Bbs[:, sl], Bp[:])
                    Cp = ps_a.tile([128, Q], f32, tag="psa")
                    nc.tensor.matmul(Cp[:], wt["crepA"][:, l * 128:(l + 1) * 128],
                                     xiA[:, sl], start=True, stop=False)
                    nc.tensor.matmul(Cp[:], wt["crepB"][:, l * 128:(l + 1) * 128],
                                     xiB[:, sl], start=False, stop=True)
                    nc.scalar.copy(Cbs[:, sl], Cp[:])

                # ============ S4-S6: grid pipeline, k-outer ==========
                # delta/u -> (e,n) grid via selector matmuls into PSUM;
                # dA = exp(A*delta_b) per chunk; scan fires as soon as its
                # tile's 4 chunks are done; hc reuses dA's slot after scan.
                y_ps = [ps_y.tile([E, Q], f32, tag="psy", name=f"yps{c}")
                        for c in range(NCH)]
                ygd = dr.tile([E, L], bf16, tag="ygd")
                for k, (pst, pc) in enumerate(GTILES):
                    sd = wt["seld"][:, k * 128:k * 128 + pc]
                    dA = gr.tile([pc, L], bf16, tag=f"dA{k}", name=f"dA{k}")
                    dBx = gr.tile([pc, L], bf16, tag=f"dBx{k}", name=f"dBx{k}")
                    for c in range(NCH):
                        sl = slice(c * Q, (c + 1) * Q)
                        db = ps_a.tile([128, Q], f32, tag="psa")
                        nc.tensor.matmul(db[0:pc, :], sd, delta[:, sl])
                        nc.scalar.activation(
                            dA[:, sl], db[0:pc, :], AF.Exp,
                            scale=wt["asc"][0:pc, l * 5 + k:l * 5 + k + 1])
                        ub = ps_a.tile([128, Q], f32, tag="psa")
                        nc.tensor.matmul(ub[0:pc, :], sd, u[:, sl])
                        nc.vector.tensor_tensor(dBx[:, sl], ub[0:pc, :],
                                                Bbs[0:pc, sl], OP.mult)
                    hg = gr.tile([pc, L], bf16, tag=f"hg{k}", name=f"hg{k}")
                    nc.vector.tensor_tensor_scan(
                        hg[:], dA[:], dBx[:], 0.0, OP.mult, OP.add)
                    hc = gr.tile([pc, L], bf16, tag=f"dA{k}", name=f"hc{k}")
                    nc.vector.tensor_tensor(hc[:], hg[:], Cbs[0:pc, :], OP.mult)
                    for c in range(NCH):
                        sl = slice(c * Q, (c + 1) * Q)
                        nc.tensor.matmul(y_ps[c][:],
                                         wt["red"][0:pc, k * E:(k + 1) * E],
                                         hc[:, sl],
                                         start=(k == 0), stop=(k == 4))

                # ================= S7: gate =================
                ygf = fl.tile([E, L], bf16, tag="ygf")
                for c in range(NCH):
                    sl = slice(c * Q, (c + 1) * Q)
                    yg1 = sm.tile([E, Q], bf16, tag="yg1", bufs=2)
                    nc.vector.scalar_tensor_tensor(
                        yg1[:], xiA[0:E, sl], wt["dpv"][:, l:l + 1], y_ps[c][:],
                        OP.mult, OP.add)
                    nc.vector.tensor_tensor(ygf[:, sl], yg1[:], zs[:, sl],
                                            OP.mult)
                nc.sync.dma_start(ygd[:], ygf[:])
                y_ps = None

                # ================= S8: AllGather =================
                yga = dr.tile([GROUP * E, L], bf16, tag="yga")
                nc.gpsimd.collective_compute(
                    "AllGather", OP.bypass,
                    replica_groups=[[0, 1, 2, 3], [4, 5, 6, 7]],
                    ins=[ygd.opt()], outs=[yga.opt()])
                yfA = fl.tile([128, L], bf16, tag="yfA")
                yfB = fl.tile([12, L], bf16, tag="yfB")
                nc.sync.dma_start(yfA[:], yga[0:128, :])
                nc.sync.dma_start(yfB[:], yga[128:ED, :])

                # ================= S9: out_proj + residual =================
                for c in range(NCH):
                    sl = slice(c * Q, (c + 1) * Q)
                    op = ps_a.tile([D, Q], f32, tag="psa")
                    nc.tensor.matmul(op[:], wt["outwA"][:, l * D:(l + 1) * D],
                                     yfA[:, sl], start=True, stop=False)
                    nc.tensor.matmul(op[:], wt["outwB"][:, l * D:(l + 1) * D],
                                     yfB[:, sl], start=False, stop=True)
                    nc.vector.tensor_tensor(h_nxt[:, sl], h_cur[:, sl], op[:],
                                            OP.add)
                h_cur, h_nxt = h_nxt, h_cur

            # ---- head ----
            for c in range(NCH):
                sl = slice(c * Q, (c + 1) * Q)
                hp = ps_s.tile([1, Q], f32, tag="pss")
                nc.tensor.matmul(hp[:], wt["wout"][:], h_cur[:, sl])
                ot = sm.tile([1, Q], f32, tag="ot")
                nc.scalar.activation(ot[:], hp[:], AF.Tanh,
                                     bias=wt["bout"][:, 0:1], scale=1.0)
                nc.sync.dma_start(out_d[:, sl], ot[:])

    nc.compile()
    return nc


def _prep_inputs(inputs):
    """Returns in_maps: list of 8 dicts (core = s*4 + j)."""
    g = {k: np.asarray(v, np.float32) for k, v in inputs.items()}
    nw, ipw = g["norm_w"], g["in_proj_w"]
    cw, cb = g["conv_w"], g["conv_b"]
    xpw, dtw, dtb = g["x_proj_w"], g["dt_w"], g["dt_b"]
    alog, dpv, opw = g["A_log"], g["D_p"], g["out_proj_w"]
    b16 = ml_dtypes.bfloat16

    maps = []
    for s in range(2):
        for j in range(4):
            own = np.arange(E * j, E * (j + 1))
            perm = np.r_[own, np.delete(np.arange(ED), own)]
            m = {
                "x_t": np.ascontiguousarray(g["x"][s].T),
                "w_in": np.ascontiguousarray(g["W_in"].T),
                "b_in": g["b_in"].reshape(D, 1),
                "dtb": np.stack([dtb[l][own] for l in range(NL)], 1),
                "dpv": np.stack([dpv[l][own] for l in range(NL)], 1),
                "ones70": np.full((D, 1), 1.0 / D, b16),
                "ones1": np.ones((1, D), b16),
                "wout": np.ascontiguousarray(g["W_out"].T),
                "bout": g["b_out"].reshape(1, 1),
            }
            taps = np.zeros((D, NL * 4 * ED), np.float32)
            zw = np.zeros((D, NL * E), np.float32)
            bw = np.zeros((ED, NL * 128), np.float32)
            cwm = np.zeros((ED, NL * 128), np.float32)
            dtwT = np.zeros((ED, NL * E), np.float32)
            outw = np.zeros((ED, NL * D), np.float32)
            cbp = np.zeros((ED, NL), np.float32)
            asc = np.zeros((128, NL * 5), np.float32)
            for l in range(NL):
                Wxi = ipw[l][:ED] * nw[l][None, :]          # (140,70)
                for k in range(4):
                    tap = (cw[l, :, 0, k:k + 1] * Wxi)[perm]
                    taps[:, (l * 4 + k) * ED:(l * 4 + k + 1) * ED] = tap.T
                zw[:, l * E:(l + 1) * E] = (ipw[l][ED:2 * ED] * nw[l][None, :])[own].T
                brep = xpw[l][DTR + (np.arange(128) % N)][:, perm]   # (128,140)
                crep = xpw[l][DTR + N + (np.arange(128) % N)][:, perm]
                bw[:, l * 128:(l + 1) * 128] = brep.T
                cwm[:, l * 128:(l + 1) * 128] = crep.T
                mdt = dtw[l][own] @ xpw[l][0:DTR]           # (35,140)
                dtwT[:, l * E:(l + 1) * E] = mdt[:, perm].T
                outw[:, l * D:(l + 1) * D] = opw[l].T
                cbp[:, l] = cb[l][perm]
                A = -np.exp(alog[l])                        # (140,16)
                Ao = A[own]                                 # (35,16)
                for k, (pst, pc) in enumerate(GTILES):
                    e0 = 8 * k
                    v = Ao[e0:e0 + pc // 16].reshape(-1)    # (pc,)
                    asc[0:pc, l * 5 + k] = v
            m.update(taps=taps.astype(b16), zw=zw.astype(b16),
                     brepA=bw[0:128].astype(b16), brepB=bw[128:ED].astype(b16),
                     crepA=cwm[0:128].astype(b16), crepB=cwm[128:ED].astype(b16),
                     dtwA=dtwT[0:128].astype(b16), dtwB=dtwT[128:ED].astype(b16),
                     outwA=outw[0:128].astype(b16), outwB=outw[128:ED].astype(b16),
                     cbA=cbp[0:128], cbB=cbp[128:ED], asc=asc)
            seld = np.zeros((E, 5 * 128), np.float32)
            red = np.zeros((128, 5 * E), np.float32)
            for k, (pst, pc) in enumerate(GTILES):
                for p in range(pc):
                    seld[8 * k + p // 16, k * 128 + p] = 1.0
            for k, (pst, pc) in enumerate(GTILES):
                for p in range(pc):
                    red[p, k * E + 8 * k + p // 16] = 1.0
            m.update(seld=seld.astype(b16), red=red.astype(b16))
            maps.append(m)
    return maps


def kernel(**inputs):
    if "nc" not in _CACHE:
        _CACHE["nc"] = _build_nc()
    nc = _CACHE["nc"]
    in_maps = _prep_inputs(inputs)
    res = run_bass_kernel_spmd(nc, in_maps, list(range(NCORES))).results
    out = np.concatenate([res[0]["out"].ravel(), res[4]["out"].ravel()])
    return out.astype(np.float32)


# revision 35
# speedup vs baseline: 1.0077x; 1.0077x over previous
"""Trainium2 Bass kernel for nn_Net_24077586661451 (12-layer Mamba, d_model=70).

Sharding: 8 cores = 2 samples x 4 e-chunks (ED=140 -> 35/core); params
replicated, one AllGather of y per layer over each 4-core group.
Per-core scan grid: 560 partitions (35 e x 16 n, e-major p = e*16+n) as 5
partition tiles (4x128 + 48). L = 2048 = 4 chunks of Q=512 (PSUM free size).

vs the 5.5ms v0: all matmuls bf16 (fp32 PE is 4 cyc/row, bf16 is 1);
full-L (FD=2048) tensor_tensor_scan per grid tile, k-pipelined with the
dA/dBx production; B/C produced pre-replicated straight from xi (B_rep row
p = x_proj row DTR + p%16, identical for every grid tile); softplus =
exp + 3-term ln1p series on DVE (input <= -1.8 so t <= 0.17); rmsnorm
rsqrt = exp(-0.5 ln(mean)); activation blocks gated through tiny DVE-made
scale operands so the act table loads ~4x/layer instead of thrashing on
the scheduler's readiness-order interleaving; one AllGather per layer
(split AGs and DMA-broadcast variants both measured slower).

Layer pipeline (per layer):
  S1 rmsnorm: h^2 (DVE), ones-matmul (PE), Ln / Exp blocks (ACT) -> hsc bf16
  S2 conv-fused in_proj (4 shifted-tap PE matmuls) + z proj, Silu (ACT)
  S3 dt (PE, premult dt_w@x_proj), exp (ACT) + ln1p series (DVE) -> delta,
     u = delta*xi (GP); B_rep/C_rep (PE) + copies (ACT) -> full-L SBUF
  S4-S6 per grid tile k: delta/u selector-matmuls -> PSUM (PE),
     dA = exp(A*delta_b) (ACT), dBx = u_b*B_rep (DVE),
     tensor_tensor_scan FD=2048 (DVE), hc = h*C_rep (DVE),
     n-reduce matmuls -> y PSUM (PE)
  S7 gate: D*xi+y (DVE stt), *silu(z) (DVE), DMA chunks to DRAM
  S8 AllGather y over the 4-core group (DRAM bounce)
  S9 out_proj (PE) + residual add (DVE f32)

Each core's xi channel order is permuted so its own 35 channels are rows 0:35
(weights permuted host-side; the program is identical across cores - SPMD).
"""
import ml_dtypes
import numpy as np

import concourse.bass as bass
import concourse.bacc as bacc
import concourse.mybir as mybir
import concourse.tile as tile
from concourse.bass_utils import run_bass_kernel_spmd

f32 = mybir.dt.float32
bf16 = mybir.dt.bfloat16
AF = mybir.ActivationFunctionType
OP = mybir.AluOpType

B, L, IN_DIM, D, ED, N, NL, DTR = 2, 2048, 32, 70, 140, 16, 12, 5
E = ED // 4                      # 35 channels per core
NCORES, GROUP = 8, 4
Q = 512
NCH = L // Q
EPS = 1e-5
# grid partition tiles: (pstart, pcount); p = e_loc*16 + n
GTILES = [(0, 128), (128, 128), (256, 128), (384, 128), (512, 48)]

_CACHE = {}


def _build_nc():
    nc = bacc.Bacc("TRN2", target_bir_lowering=False, debug=False)

    di = {}

    def dram_in(name, shape, dt=f32):
        di[name] = nc.dram_tensor(name, list(shape), dt, kind="ExternalInput")
        return di[name]

    dram_in("x_t", (IN_DIM, L))
    dram_in("w_in", (IN_DIM, D))
    dram_in("b_in", (D, 1))
    dram_in("taps", (D, NL * 4 * ED), bf16)
    dram_in("zw", (D, NL * E), bf16)
    dram_in("brepA", (128, NL * 128), bf16)
    dram_in("brepB", (12, NL * 128), bf16)
    dram_in("crepA", (128, NL * 128), bf16)
    dram_in("crepB", (12, NL * 128), bf16)
    dram_in("dtwA", (128, NL * E), bf16)
    dram_in("dtwB", (12, NL * E), bf16)
    dram_in("outwA", (128, NL * D), bf16)
    dram_in("outwB", (12, NL * D), bf16)
    dram_in("dtb", (E, NL))
    dram_in("cbA", (128, NL))
    dram_in("cbB", (12, NL))
    dram_in("dpv", (E, NL))
    dram_in("asc", (128, NL * 5))
    dram_in("seld", (E, 5 * 128), bf16)
    dram_in("red", (128, 5 * E), bf16)
    dram_in("ones70", (D, 1), bf16)
    dram_in("ones1", (1, D), bf16)
    dram_in("wout", (D, 1))
    dram_in("bout", (1, 1))
    out_d = nc.dram_tensor("out", [1, L], f32, kind="ExternalOutput")

    with tile.TileContext(nc) as tc:
        with (
            tc.tile_pool(name="wts", bufs=1) as wts,
            tc.tile_pool(name="hbuf", bufs=1) as hbuf,
            tc.tile_pool(name="fl", bufs=1) as fl,           # full-L per layer
            tc.tile_pool(name="gr", bufs=1) as gr,           # grid full-L
            tc.tile_pool(name="sm", bufs=3) as sm,           # per-chunk small
            tc.tile_pool(name="ps_a", bufs=3, space="PSUM") as ps_a,
            tc.tile_pool(name="ps_y", bufs=4, space="PSUM") as ps_y,
            tc.tile_pool(name="ps_s", bufs=1, space="PSUM") as ps_s,
            tc.tile_pool(name="dr", bufs=2, space="DRAM") as dr,
        ):
            wt = {}
            for name, h in di.items():
                t = wts.tile(list(h.shape), h.dtype, tag=f"w_{name}")
                nc.sync.dma_start(t[:], h[:])
                wt[name] = t

            # persistent activation buffers
            h_a = hbuf.tile([D, L], f32)
            h_b = hbuf.tile([D, L], f32)
            hsc = hbuf.tile([D, L + 3], bf16)  # rms-scaled h, 3-col zero pad
            nc.vector.memset(hsc[:, 0:3], 0.0)

            # ---- embed: h_a = W_in @ x + b_in ----
            for c in range(NCH):
                sl = slice(c * Q, (c + 1) * Q)
                h0 = ps_a.tile([D, Q], f32, tag="psa")
                nc.tensor.matmul(h0[:], wt["w_in"][:], wt["x_t"][:, sl])
                nc.scalar.activation(h_a[:, sl], h0[:], AF.Identity,
                                     bias=wt["b_in"][:, 0:1], scale=1.0)

            h_cur, h_nxt = h_a, h_b

            for l in range(NL):
                # ================= S1: rmsnorm =================
                # rsqrt via exp(-0.5*ln(mean)); all Ln writes land in one
                # full-L tile so the Exp block only becomes ready after the
                # last Ln - whole-tile deps keep the act-table blocks grouped.
                rsf = fl.tile([1, L], bf16, tag="rsf")
                lnf = fl.tile([1, L], f32, tag="lnf")
                for c in range(NCH):
                    sl = slice(c * Q, (c + 1) * Q)
                    sq = sm.tile([D, Q], bf16, tag="sq", bufs=2)
                    nc.vector.tensor_tensor(sq[:], h_cur[:, sl], h_cur[:, sl],
                                            OP.mult)
                    ms = ps_s.tile([1, Q], f32, tag="pss")
                    nc.tensor.matmul(ms[:], wt["ones70"][:], sq[:])
                    nc.scalar.activation(lnf[:, sl], ms[:], AF.Ln)
                # scale operand built from the LAST Ln output: the whole Exp
                # block waits for the Ln block, so the act table loads once
                # per block instead of thrashing on ln<->exp alternation.
                g1 = sm.tile([1, 1], f32, tag="g1", bufs=2)
                nc.vector.tensor_scalar(g1[:], lnf[:, L - 1:L], 0.0, -0.5,
                                        OP.mult, OP.add)
                for c in range(NCH):
                    sl = slice(c * Q, (c + 1) * Q)
                    nc.scalar.activation(rsf[:, sl], lnf[:, sl], AF.Exp,
                                         scale=g1[:])
                for c in range(NCH):
                    sl = slice(c * Q, (c + 1) * Q)
                    rs70 = ps_a.tile([D, Q], f32, tag="psa")
                    nc.tensor.matmul(rs70[:], wt["ones1"][:], rsf[:, sl])
                    nc.vector.tensor_tensor(hsc[:, 3 + c * Q:3 + (c + 1) * Q],
                                            h_cur[:, sl], rs70[:], OP.mult)

                # ================= S2: in_proj taps + z =================
                xiA = fl.tile([128, L], bf16, tag="xiA")
                xiB = fl.tile([12, L], bf16, tag="xiB")
                zs = fl.tile([E, L], bf16, tag="zs")
                for c in range(NCH):
                    sl = slice(c * Q, (c + 1) * Q)
                    xa = ps_a.tile([128, Q], f32, tag="psa")
                    xb = ps_s.tile([12, Q], f32, tag="pss")
                    for k in range(4):
                        tap = wt["taps"][:, (l * 4 + k) * ED:(l * 4 + k + 1) * ED]
                        rhs = hsc[:, c * Q + k:c * Q + k + Q]
                        nc.tensor.matmul(xa[:], tap[:, 0:128], rhs,
                                         start=(k == 0), stop=(k == 3))
                        nc.tensor.matmul(xb[:], tap[:, 128:ED], rhs,
                                         start=(k == 0), stop=(k == 3))
                    zp = ps_s.tile([E, Q], f32, tag="pss")
                    nc.tensor.matmul(zp[:], wt["zw"][:, l * E:(l + 1) * E],
                                     hsc[:, 3 + c * Q:3 + (c + 1) * Q])
                    nc.scalar.activation(xiA[:, sl], xa[:], AF.Silu,
                                         bias=wt["cbA"][:, l:l + 1], scale=1.0)
                    nc.scalar.activation(xiB[:, sl], xb[:], AF.Silu,
                                         bias=wt["cbB"][:, l:l + 1], scale=1.0)
                    nc.scalar.activation(zs[:, sl], zp[:], AF.Silu)

                # ================= S3: delta (softplus), u, B/C reps =========
                delta = fl.tile([E, L], bf16, tag="delta")
                u = fl.tile([E, L], bf16, tag="u")
                dps, ezs = [], []
                for c in range(NCH):
                    sl = slice(c * Q, (c + 1) * Q)
                    dp = ps_a.tile([E, Q], f32, tag="psa")
                    nc.tensor.matmul(dp[:], wt["dtwA"][:, l * E:(l + 1) * E],
                                     xiA[:, sl], start=True, stop=False)
                    nc.tensor.matmul(dp[:], wt["dtwB"][:, l * E:(l + 1) * E],
                                     xiB[:, sl], start=False, stop=True)
                    dps.append(dp)
                g2 = sm.tile([E, 1], f32, tag="g2", bufs=2)
                nc.vector.tensor_scalar(g2[:], zs[:, L - 1:L], 0.0, 1.0,
                                        OP.mult, OP.add)
                for c in range(NCH):
                    ez = sm.tile([E, Q], bf16, tag="ez", bufs=3)
                    nc.scalar.activation(ez[:], dps[c][:], AF.Exp,
                                         bias=wt["dtb"][:, l:l + 1],
                                         scale=g2[:])
                    ezs.append(ez)
                for c in range(NCH):
                    # softplus(x) = ln(1+t), t=exp(x) <= 0.17 here (dt_b in
                    # [-4,-2]): 3-term series t*(1 - t*(0.5 - t/3)) on DVE
                    # keeps ln off the scalar engine (act-table thrash).
                    sl = slice(c * Q, (c + 1) * Q)
                    t = ezs[c]
                    p1 = sm.tile([E, Q], bf16, tag="p1", bufs=2)
                    nc.vector.tensor_scalar(p1[:], t[:], -1.0 / 3, 0.5,
                                            OP.mult, OP.add)
                    p2 = sm.tile([E, Q], bf16, tag="p2", bufs=2)
                    nc.vector.tensor_tensor(p2[:], t[:], p1[:], OP.mult)
                    p3 = sm.tile([E, Q], bf16, tag="p1", bufs=2, name="p3")
                    nc.vector.tensor_scalar(p3[:], p2[:], -1.0, 1.0,
                                            OP.mult, OP.add)
                    nc.vector.tensor_tensor(delta[:, sl], t[:], p3[:], OP.mult)
                    nc.gpsimd.tensor_tensor(u[:, sl], delta[:, sl],
                                            xiA[0:E, sl], OP.mult)
                dps = ezs = None
                # B_rep[p,t] = B[p%16,t] identical for every grid tile; full-L
                Bbs = fl.tile([128, L], bf16, tag="Bbs")
                Cbs = fl.tile([128, L], bf16, tag="Cbs")
                for c in range(NCH):
                    sl = slice(c * Q, (c + 1) * Q)
                    Bp = ps_a.tile([128, Q], f32, tag="psa")
                    nc.tensor.matmul(Bp[:], wt["brepA"][:, l * 128:(l + 1) * 128],
                                     xiA[:, sl], start=True, stop=False)
                    nc.tensor.matmul(Bp[:], wt["brepB"][:, l * 128:(l + 1) * 128],
                                     xiB[:, sl], start=False, stop=True)
                    nc.scalar.copy(Bbs[:, sl], Bp[:])
                    Cp = ps_a.tile([128, Q], f32, tag="psa")
                    nc.tensor.matmul(Cp[:], wt["crepA"][:, l * 128:(l + 1) * 128],
                                     xiA[:, sl], start=True, stop=False)
                    nc.tensor.matmul(Cp[:], wt["crepB"][:, l * 128:(l + 1) * 128],
                                     xiB[:, sl], start=False, stop=True)
                    nc.scalar.copy(Cbs[:, sl], Cp[:])

                # ============ S4-S6: grid pipeline, k-outer ==========
                # delta/u -> (e,n) grid via selector matmuls into PSUM;
                # dA = exp(A*delta_b) per chunk; scan fires as soon as its
                # tile's 4 chunks are done; hc reuses dA's slot after scan.
                y_ps = [ps_y.tile([E, Q], f32, tag="psy", name=f"yps{c}")
                        for c in range(NCH)]
                ygd = dr.tile([E, L], bf16, tag="ygd")
                for k, (pst, pc) in enumerate(GTILES):
                    sd = wt["seld"][:, k * 128:k * 128 + pc]
                    dA = gr.tile([pc, L], bf16, tag=f"dA{k}", name=f"dA{k}")
                    dBx = gr.tile([pc, L], bf16, tag=f"dBx{k}", name=f"dBx{k}")
                    for c in range(NCH):
                        sl = slice(c * Q, (c + 1) * Q)
                        db = ps_a.tile([128, Q], f32, tag="psa")
                        nc.tensor.matmul(db[0:pc, :], sd, delta[:, sl])
                        nc.scalar.activation(
                            dA[:, sl], db[0:pc, :], AF.Exp,
                            scale=wt["asc"][0:pc, l * 5 + k:l * 5 + k + 1])
                        ub = ps_a.tile([128, Q], f32, tag="psa")
                        nc.tensor.matmul(ub[0:pc, :], sd, u[:, sl])
                        nc.vector.tensor_tensor(dBx[:, sl], ub[0:pc, :],
                                                Bbs[0:pc, sl], OP.mult)
                    hg = gr.tile([pc, L], bf16, tag=f"hg{k}", name=f"hg{k}")
                    nc.vector.tensor_tensor_scan(
                        hg[:], dA[:], dBx[:], 0.0, OP.mult, OP.add)
                    hc = gr.tile([pc, L], bf16, tag=f"dA{k}", name=f"hc{k}")
                    nc.vector.tensor_tensor(hc[:], hg[:], Cbs[0:pc, :], OP.mult)
                    for c in range(NCH):
                        sl = slice(c * Q, (c + 1) * Q)
                        nc.tensor.matmul(y_ps[c][:],
                                         wt["red"][0:pc, k * E:(k + 1) * E],
                                         hc[:, sl],
                                         start=(k == 0), stop=(k == 4))

                # ================= S7: gate =================
                ygf = fl.tile([E, L], bf16, tag="ygf")
                for c in range(NCH):
                    sl = slice(c * Q, (c + 1) * Q)
                    yg1 = sm.tile([E, Q], bf16, tag="yg1", bufs=2)
                    nc.vector.scalar_tensor_tensor(
                        yg1[:], xiA[0:E, sl], wt["dpv"][:, l:l + 1], y_ps[c][:],
                        OP.mult, OP.add)
                    nc.vector.tensor_tensor(ygf[:, sl], yg1[:], zs[:, sl],
                                            OP.mult)
                nc.sync.dma_start(ygd[:], ygf[:])
                y_ps = None

                # ================= S8: AllGather =================
                yga = dr.tile([GROUP * E, L], bf16, tag="yga")
                nc.gpsimd.collective_compute(
                    "AllGather", OP.bypass,
                    replica_groups=[[0, 1, 2, 3], [4, 5, 6, 7]],
                    ins=[ygd.opt()], outs=[yga.opt()])
                yfA = fl.tile([128, L], bf16, tag="yfA")
                yfB = fl.tile([12, L], bf16, tag="yfB")
                nc.sync.dma_start(yfA[:], yga[0:128, :])
                nc.sync.dma_start(yfB[:], yga[128:ED, :])

                # ================= S9: out_proj + residual =================
                for c in range(NCH):
                    sl = slice(c * Q, (c + 1) * Q)
                    op = ps_a.tile([D, Q], f32, tag="psa")
                    nc.tensor.matmul(op[:], wt["outwA"][:, l * D:(l + 1) * D],
                                     yfA[:, sl], start=True, stop=False)
                    nc.tensor.matmul(op[:], wt["outwB"][:, l * D:(l + 1) * D],
                                     yfB[:, sl], start=False, stop=True)
                    nc.vector.tensor_tensor(h_nxt[:, sl], h_cur[:, sl], op[:],
                                            OP.add)
                h_cur, h_nxt = h_nxt, h_cur

            # ---- head ----
            for c in range(NCH):
                sl = slice(c * Q, (c + 1) * Q)
                hp = ps_s.tile([1, Q], f32, tag="pss")
                nc.tensor.matmul(hp[:], wt["wout"][:], h_cur[:, sl])
                ot = sm.tile([1, Q], f32, tag="ot")
                nc.scalar.activation(ot[:], hp[:], AF.Tanh,
                                     bias=wt["bout"][:, 0:1], scale=1.0)
                nc.sync.dma_start(out_d[:, sl], ot[:])

    nc.compile()
    return nc


def _prep_inputs(inputs):
    """Returns in_maps: list of 8 dicts (core = s*4 + j)."""
    g = {k: np.asarray(v, np.float32) for k, v in inputs.items()}
    nw, ipw = g["norm_w"], g["in_proj_w"]
    cw, cb = g["conv_w"], g["conv_b"]
    xpw, dtw, dtb = g["x_proj_w"], g["dt_w"], g["dt_b"]
    alog, dpv, opw = g["A_log"], g["D_p"], g["out_proj_w"]
    b16 = ml_dtypes.bfloat16

    maps = []
    for s in range(2):
        for j in range(4):
            own = np.arange(E * j, E * (j + 1))
            perm = np.r_[own, np.delete(np.arange(ED), own)]
            m = {
                "x_t": np.ascontiguousarray(g["x"][s].T),
                "w_in": np.ascontiguousarray(g["W_in"].T),
                "b_in": g["b_in"].reshape(D, 1),
                "dtb": np.stack([dtb[l][own] for l in range(NL)], 1),
                "dpv": np.stack([dpv[l][own] for l in range(NL)], 1),
                "ones70": np.full((D, 1), 1.0 / D, b16),
                "ones1": np.ones((1, D), b16),
                "wout": np.ascontiguousarray(g["W_out"].T),
                "bout": g["b_out"].reshape(1, 1),
            }
            taps = np.zeros((D, NL * 4 * ED), np.float32)
            zw = np.zeros((D, NL * E), np.float32)
            bw = np.zeros((ED, NL * 128), np.float32)
            cwm = np.zeros((ED, NL * 128), np.float32)
            dtwT = np.zeros((ED, NL * E), np.float32)
            outw = np.zeros((ED, NL * D), np.float32)
            cbp = np.zeros((ED, NL), np.float32)
            asc = np.zeros((128, NL * 5), np.float32)
            for l in range(NL):
                Wxi = ipw[l][:ED] * nw[l][None, :]          # (140,70)
                for k in range(4):
                    tap = (cw[l, :, 0, k:k + 1] * Wxi)[perm]
                    taps[:, (l * 4 + k) * ED:(l * 4 + k + 1) * ED] = tap.T
                zw[:, l * E:(l + 1) * E] = (ipw[l][ED:2 * ED] * nw[l][None, :])[own].T
                brep = xpw[l][DTR + (np.arange(128) % N)][:, perm]   # (128,140)
                crep = xpw[l][DTR + N + (np.arange(128) % N)][:, perm]
                bw[:, l * 128:(l + 1) * 128] = brep.T
                cwm[:, l * 128:(l + 1) * 128] = crep.T
                mdt = dtw[l][own] @ xpw[l][0:DTR]           # (35,140)
                dtwT[:, l * E:(l + 1) * E] = mdt[:, perm].T
                outw[:, l * D:(l + 1) * D] = opw[l].T
                cbp[:, l] = cb[l][perm]
                A = -np.exp(alog[l])                        # (140,16)
                Ao = A[own]                                 # (35,16)
                for k, (pst, pc) in enumerate(GTILES):
                    e0 = 8 * k
                    v = Ao[e0:e0 + pc // 16].reshape(-1)    # (pc,)
                    asc[0:pc, l * 5 + k] = v
            m.update(taps=taps.astype(b16), zw=zw.astype(b16),
                     brepA=bw[0:128].astype(b16), brepB=bw[128:ED].astype(b16),
                     crepA=cwm[0:128].astype(b16), crepB=cwm[128:ED].astype(b16),
                     dtwA=dtwT[0:128].astype(b16), dtwB=dtwT[128:ED].astype(b16),
                     outwA=outw[0:128].astype(b16), outwB=outw[128:ED].astype(b16),
                     cbA=cbp[0:128], cbB=cbp[128:ED], asc=asc)
            seld = np.zeros((E, 5 * 128), np.float32)
            red = np.zeros((128, 5 * E), np.float32)
            for k, (pst, pc) in enumerate(GTILES):
                for p in range(pc):
                    seld[8 * k + p // 16, k * 128 + p] = 1.0
            for k, (pst, pc) in enumerate(GTILES):
                for p in range(pc):
                    red[p, k * E + 8 * k + p // 16] = 1.0
            m.update(seld=seld.astype(b16), red=red.astype(b16))
            maps.append(m)
    return maps


def kernel(**inputs):
    if "nc" not in _CACHE:
        _CACHE["nc"] = _build_nc()
    nc = _CACHE["nc"]
    in_maps = _prep_inputs(inputs)
    res = run_bass_kernel_spmd(nc, in_maps, list(range(NCORES))).results
    out = np.concatenate([res[0]["out"].ravel(), res[4]["out"].ravel()])
    return out.astype(np.float32)
